# revision 3
# baseline (speedup 1.0000x reference)
"""Trainium2 Bass kernel for nn_EnsembleConvRNN.

Computation: LayerNorm -> depthwise causal conv1d (K=4) -> broadcast to E
ensemble members -> 4 stacked RNN layers, each: input GEMM (batch-ensemble
scaled) + sequential tanh scan over L=256.

Sharding: data-parallel over batch B (4 of 32 per core, all E=8 ensemble
members local), weights replicated; the sequential scan stays local per
shard.  Device layout keeps hidden dim H on partitions (2 chunks of 128)
and (l, be) on the free axis.  The scan runs as a 4-layer software
wavefront (lag G steps between layers) so each layer's matmul->tanh serial
chain hides behind the other layers.

PSUM plan: each layer owns 2 banks, each holding pre-activations for 8
consecutive steps as [mc, t%8, be].  For layers 1-3 the input GEMM writes
pre DIRECTLY into the bank (matmul, start=True on first), and the per-step
recurrent matmuls accumulate on top with start=False (has_written set by
the GEMM fill).  Layer 0's pre (per-ensemble folded weights) is computed
upfront into SBUF fp16 and DVE-bulk-copied into its banks 8 steps at a
time; a one-time dummy matmul sets those banks' has_written bits so
start=False accumulation works there too.  Each step then costs only
4 matmuls + 1 tanh activation on the critical chain.
"""

import os
import numpy as np

P = 128
B, L, D, H, E, NL, K = 32, 256, 256, 256, 8, 4, 4
NCORES = 8
BPC = B // NCORES          # b per core = 4
BE = BPC * E               # 32 scan rows per core, be = e*BPC + b
KC = D // P                # 2
MC = H // P                # 2
G = 16                     # wavefront lag between layers (steps)
SB = 8                     # steps per psum bank
NSB = L // SB              # 32
OB = 16                    # output dma block (steps)
HW = 48                    # h rolling-window length (steps)

_CACHE = {}
last_run_info = {}


def _build(per_e_gemm: bool, has_bias: bool):
    import concourse.tile as tile
    import concourse.mybir as mybir
    from concourse import bacc
    from concourse.masks import make_identity

    dt = mybir.dt
    AF = mybir.ActivationFunctionType
    ALU = mybir.AluOpType

    nc = bacc.Bacc(
        "TRN2", target_bir_lowering=False, debug=False, enable_asserts=False
    )

    xs = nc.dram_tensor("xs", [BPC, L, D], dt.float32, kind="ExternalInput").ap()
    wf0 = nc.dram_tensor("wf0", [P, E, KC, H], dt.float16, kind="ExternalInput").ap()
    if per_e_gemm:
        wih = nc.dram_tensor(
            "wih", [P, NL - 1, E, KC, H], dt.float16, kind="ExternalInput"
        ).ap()
    else:
        wih = nc.dram_tensor(
            "wih", [P, NL - 1, KC, H], dt.float16, kind="ExternalInput"
        ).ap()
    whh = nc.dram_tensor("whh", [P, NL, KC, MC, P], dt.float16, kind="ExternalInput").ap()
    cwb = nc.dram_tensor("cwb", [P, KC, K + 1], dt.float32, kind="ExternalInput").ap()
    lngb = nc.dram_tensor("lngb", [P, KC, 2], dt.float32, kind="ExternalInput").ap()
    biasd = nc.dram_tensor("biasd", [P, NL, MC], dt.float32, kind="ExternalInput").ap()
    outp = nc.dram_tensor("outp", [P, MC, L, BE], dt.float32, kind="ExternalOutput").ap()
    hlast = nc.dram_tensor("hlast", [P, MC, BE], dt.float32, kind="ExternalOutput").ap()

    with tile.TileContext(nc) as tc:
        with tc.tile_pool(name="persist", bufs=1) as persist:
            t_wf0 = persist.tile([P, E, KC, H], dt.float16, tag="wf0", name="wf0")
            nc.sync.dma_start(t_wf0[:], wf0)
            if per_e_gemm:
                t_wih = persist.tile(
                    [P, NL - 1, E, KC, H], dt.float16, tag="wih", name="wih"
                )
            else:
                t_wih = persist.tile(
                    [P, NL - 1, KC, H], dt.float16, tag="wih", name="wih"
                )
            nc.sync.dma_start(t_wih[:], wih)
            t_whh = persist.tile([P, NL, KC, MC, P], dt.float16, tag="whh", name="whh")
            nc.sync.dma_start(t_whh[:], whh)
            t_cwb = persist.tile([P, KC, K + 1], dt.float32, tag="cwb", name="cwb")
            nc.sync.dma_start(t_cwb[:], cwb)
            t_lngb = persist.tile([P, KC, 2], dt.float32, tag="lngb", name="lngb")
            nc.sync.dma_start(t_lngb[:], lngb)
            t_bias = persist.tile([P, NL, MC], dt.float32, tag="biasd", name="biasd")
            nc.sync.dma_start(t_bias[:], biasd)
            t_eps = persist.tile([P, 1], dt.float32, tag="eps", name="eps")
            nc.vector.memset(t_eps[:], 1e-5)
            ident = persist.tile([P, P], dt.float32, tag="ident", name="ident")
            make_identity(nc, ident[:])

            conv16 = persist.tile([P, KC, L, BPC], dt.float16, tag="conv16", name="conv16")
            xpad = persist.tile([P, KC, BPC, K - 1 + L], dt.float32, tag="xpad", name="xpad")
            # layer-0 pre, grouped by 8-step psum blocks: [blk, mc, t, be]
            pre0 = persist.tile([P, NSB, MC, SB, BE], dt.float16, tag="pre0", name="pre0")
            hbuf = [
                persist.tile([P, MC, HW, BE], dt.float16, tag=f"h{i}", name=f"h{i}")
                for i in range(NL)
            ]

            # ---- LayerNorm + transpose + causal depthwise conv ----
            with (
                tc.tile_pool(name="lnp", bufs=3) as lnp,
                tc.tile_pool(name="tpp", bufs=2, space="PSUM") as tpp,
            ):
                nc.vector.memset(xpad[:, :, :, 0 : K - 1], 0.0)
                for b in range(BPC):
                    for j in range(2):
                        xt = lnp.tile([P, D], dt.float32, tag="xt", name="xt")
                        nc.sync.dma_start(xt[:], xs[b, 128 * j : 128 * (j + 1), :])
                        st = lnp.tile([P, 6], dt.float32, tag="st", name="st")
                        nc.vector.bn_stats(st[:], xt[:])
                        mv = lnp.tile([P, 2], dt.float32, tag="mv", name="mv")
                        nc.vector.bn_aggr(mv[:], st[:])
                        rstd = lnp.tile([P, 1], dt.float32, tag="rstd", name="rstd")
                        nc.scalar.activation(rstd[:], mv[:, 1:2], AF.Sqrt, bias=t_eps[:])
                        nc.vector.reciprocal(rstd[:], rstd[:])
                        xn = lnp.tile([P, D], dt.float32, tag="xn", name="xn")
                        nc.vector.scalar_tensor_tensor(
                            xn[:],
                            xt[:],
                            mv[:, 0:1],
                            rstd[:].to_broadcast((P, D)),
                            ALU.subtract,
                            ALU.mult,
                        )
                        for c in range(KC):
                            pt = tpp.tile([P, P], dt.float32, tag="tp", name="tp")
                            nc.tensor.transpose(
                                pt[:], xn[:, c * 128 : (c + 1) * 128], ident[:]
                            )
                            nc.vector.tensor_scalar(
                                xpad[:, c, b, K - 1 + 128 * j : K - 1 + 128 * (j + 1)],
                                pt[:],
                                t_lngb[:, c, 0:1],
                                t_lngb[:, c, 1:2],
                                ALU.mult,
                                ALU.add,
                            )
                for c in range(KC):
                    acc = lnp.tile([P, L, BPC], dt.float32, tag="acc", name="acc")
                    src0 = xpad[:, c, :, 0:L].rearrange("p b l -> p l b")
                    nc.vector.tensor_scalar_mul(acc[:], src0, t_cwb[:, c, 0:1])
                    for k in range(1, K):
                        srck = xpad[:, c, :, k : k + L].rearrange("p b l -> p l b")
                        nc.vector.scalar_tensor_tensor(
                            acc[:], srck, t_cwb[:, c, k : k + 1], acc[:],
                            ALU.mult, ALU.add,
                        )
                    nc.vector.tensor_scalar_add(
                        conv16[:, c], acc[:], t_cwb[:, c, K : K + 1]
                    )

            # ---- layer-0 GEMM upfront (per-ensemble folded weights) ----
            with tc.tile_pool(name="gps0", bufs=2, space="PSUM") as gps0:
                for nt in range(2):
                    for e in range(E):
                        for mc in range(MC):
                            pg = gps0.tile([P, 512], dt.float32, tag="g", name="g")
                            pgl = pg[:].rearrange("p (l b) -> p l b", b=BPC)
                            for kc in range(KC):
                                nc.tensor.matmul(
                                    pgl,
                                    t_wf0[:, e, kc, mc * 128 : (mc + 1) * 128],
                                    conv16[:, kc, nt * 128 : (nt + 1) * 128, :],
                                    start=(kc == 0),
                                    stop=(kc == KC - 1),
                                )
                            pgv = pg[:].rearrange("p (a t b) -> p a t b", t=SB, b=BPC)
                            nc.vector.tensor_scalar_add(
                                pre0[
                                    :, nt * 16 : (nt + 1) * 16, mc, :,
                                    e * BPC : (e + 1) * BPC,
                                ],
                                pgv,
                                t_bias[:, 0, mc : mc + 1],
                            )

            # ---- main phase: 4-layer wavefront scan + streamed GEMMs ----
            with (
                tc.tile_pool(name="sps", bufs=1, space="PSUM") as sps,
                tc.tile_pool(name="outs", bufs=2) as outs,
            ):
                spt = [
                    [
                        sps.tile(
                            [P, MC, SB, BE], dt.float32,
                            tag=f"sp{i}_{par}", name=f"sp{i}_{par}",
                        )
                        for par in range(2)
                    ]
                    for i in range(NL)
                ]

                def emit_bulk_pre0(blk):
                    nc.vector.tensor_copy(spt[0][blk % 2][:], pre0[:, blk])

                def emit_gemm_block(i, blk):
                    bank = spt[i][blk % 2]
                    w0 = (blk * SB) % HW
                    if not per_e_gemm:
                        first = True
                        for mc in range(MC):
                            for kc in range(KC):
                                nc.tensor.matmul(
                                    bank[:, mc],
                                    t_wih[:, i - 1, kc, mc * 128 : (mc + 1) * 128],
                                    hbuf[i - 1][:, kc, w0 : w0 + SB, :],
                                    start=first,
                                    stop=(mc == MC - 1 and kc == KC - 1),
                                    skip_group_check=True,
                                )
                                first = False
                    else:
                        first = True
                        for e in range(E):
                            for mc in range(MC):
                                for kc in range(KC):
                                    nc.tensor.matmul(
                                        bank[:, mc, :, e * BPC : (e + 1) * BPC],
                                        t_wih[
                                            :, i - 1, e, kc, mc * 128 : (mc + 1) * 128
                                        ],
                                        hbuf[i - 1][
                                            :, kc, w0 : w0 + SB,
                                            e * BPC : (e + 1) * BPC,
                                        ],
                                        start=first,
                                        stop=(
                                            e == E - 1 and mc == MC - 1 and kc == KC - 1
                                        ),
                                        skip_group_check=True,
                                    )
                                    first = False
                    if has_bias:
                        for mc in range(MC):
                            nc.vector.tensor_scalar_add(
                                bank[:, mc], bank[:, mc], t_bias[:, i, mc : mc + 1]
                            )

                def emit_scan_step(i, t):
                    bank = spt[i][(t // SB) % 2]
                    if t > 0:
                        for mc in range(MC):
                            for kc in range(KC):
                                nc.tensor.matmul(
                                    bank[:, mc, t % SB, :],
                                    t_whh[:, i, kc, mc, :],
                                    hbuf[i][:, kc, (t - 1) % HW, :],
                                    start=False,
                                    stop=(mc == MC - 1 and kc == KC - 1),
                                    skip_group_check=True,
                                )
                    nc.scalar.activation(
                        hbuf[i][:, :, t % HW, :], bank[:, :, t % SB, :], AF.Tanh
                    )

                def emit_out_block(ob):
                    stg = outs.tile([P, MC, OB, BE], dt.float32, tag="stg", name="stg")
                    w0 = (ob * OB) % HW
                    nc.vector.tensor_copy(stg[:], hbuf[NL - 1][:, :, w0 : w0 + OB, :])
                    nc.sync.dma_start(outp[:, :, ob * OB : (ob + 1) * OB, :], stg[:])

                # layer-0 warmup: dummy matmuls set has_written on both banks
                for par in range(2):
                    nc.tensor.matmul(
                        spt[0][par][:],
                        t_whh[:, 0, 0, 0, :],
                        conv16[:, 0, 0:128, :],
                        start=True,
                        stop=True,
                        skip_group_check=True,
                    )
                emit_bulk_pre0(0)
                emit_bulk_pre0(1)

                NT = L + G * (NL - 1)
                for tick in range(NT):
                    for i in range(NL):
                        t = tick - G * i
                        if not (0 <= t < L):
                            continue
                        emit_scan_step(i, t)
                        if t % SB == SB - 1:
                            blk = t // SB
                            if i < NL - 1 and blk < 2:
                                emit_gemm_block(i + 1, blk)
                            if blk + 2 < NSB:
                                if i == 0:
                                    emit_bulk_pre0(blk + 2)
                                else:
                                    emit_gemm_block(i, blk + 2)
                        if i == NL - 1 and t % OB == OB - 1:
                            emit_out_block(t // OB)

                stg2 = outs.tile([P, MC, BE], dt.float32, tag="stg2", name="stg2")
                nc.vector.tensor_copy(stg2[:], hbuf[NL - 1][:, :, (L - 1) % HW, :])
                nc.sync.dma_start(hlast, stg2[:])

    nc.compile()
    return nc


def _pack_inputs(x, conv_w, conv_b, ln_g, ln_b, W_ih, W_hh, r, s, b, per_e_gemm):
    f32, f16 = np.float32, np.float16
    # layer-0 folded input weights: wf0[p, e, kc, h] = W_ih0[h, f] * r0[e, f] * s0[e, h]
    Wf = np.einsum("hf,ef,eh->efh", W_ih[0], r[0], s[0])  # (E, D, H)
    wf0 = np.ascontiguousarray(
        Wf.reshape(E, KC, P, H).transpose(2, 0, 1, 3).astype(f16)
    )
    if per_e_gemm:
        Wr = np.einsum("ihf,ief,ieh->iefh", W_ih[1:], r[1:], s[1:])
        wih = np.ascontiguousarray(
            Wr.reshape(NL - 1, E, KC, P, H).transpose(3, 0, 1, 2, 4).astype(f16)
        )
    else:
        wih = np.ascontiguousarray(
            W_ih[1:].transpose(0, 2, 1).reshape(NL - 1, KC, P, H)
            .transpose(2, 0, 1, 3).astype(f16)
        )
    # whh[p, i, kc, mc, h'] = W_hh[i][mc*128+h', kc*128+p]
    whh = np.ascontiguousarray(
        W_hh.reshape(NL, MC, P, KC, P).transpose(4, 0, 3, 1, 2).astype(f16)
    )
    cwb = np.zeros((P, KC, K + 1), f32)
    cwb[:, :, :K] = conv_w[:, 0, :].reshape(KC, P, K).transpose(1, 0, 2)
    cwb[:, :, K] = conv_b.reshape(KC, P).T
    lngb = np.stack(
        [ln_g.reshape(KC, P).T, ln_b.reshape(KC, P).T], axis=-1
    ).astype(f32)
    biasd = np.ascontiguousarray(
        b.reshape(NL, MC, P).transpose(2, 0, 1).astype(f32)
    )
    return dict(wf0=wf0, wih=wih, whh=whh, cwb=np.ascontiguousarray(cwb),
                lngb=np.ascontiguousarray(lngb), biasd=biasd)


def kernel(x, conv_w, conv_b, ln_g, ln_b, W_ih, W_hh, r, s, b):
    from concourse.bass_utils import run_bass_kernel_spmd

    x = np.asarray(x, np.float32)
    conv_w = np.asarray(conv_w, np.float32)
    conv_b = np.asarray(conv_b, np.float32)
    ln_g = np.asarray(ln_g, np.float32)
    ln_b = np.asarray(ln_b, np.float32)
    W_ih = np.asarray(W_ih, np.float32)
    W_hh = np.asarray(W_hh, np.float32)
    r = np.asarray(r, np.float32)
    s = np.asarray(s, np.float32)
    b = np.asarray(b, np.float32)

    per_e_gemm = not (np.all(r[1:] == 1.0) and np.all(s[1:] == 1.0))
    has_bias = bool(np.any(b != 0.0))

    key = ("nc", per_e_gemm, has_bias)
    if key not in _CACHE:
        _CACHE[key] = _build(per_e_gemm, has_bias)
    nc = _CACHE[key]

    packed = _pack_inputs(
        x, conv_w, conv_b, ln_g, ln_b, W_ih, W_hh, r, s, b, per_e_gemm
    )
    in_maps = [
        dict(xs=np.ascontiguousarray(x[c * BPC : (c + 1) * BPC]), **packed)
        for c in range(NCORES)
    ]

    trace = bool(int(os.environ.get("KBENCH_TRACE", "0")))
    res = run_bass_kernel_spmd(
        nc, in_maps, core_ids=list(range(NCORES)), trace=trace
    )
    last_run_info["exec_time_ns"] = res.exec_time_ns
    last_run_info["trace"] = res.instructions_and_trace
    last_run_info["nc"] = nc
    last_run_info["in_maps"] = in_maps

    h_full = np.empty((B, E, L, H), np.float32)
    hl_full = np.empty((B, E, H), np.float32)
    for c in range(NCORES):
        arr = res.results[c]["outp"]  # (P, MC, L, BE)
        t = arr.transpose(3, 2, 1, 0).reshape(E, BPC, L, H)
        h_full[c * BPC : (c + 1) * BPC] = t.transpose(1, 0, 2, 3)
        al = res.results[c]["hlast"]  # (P, MC, BE)
        tl = al.transpose(2, 1, 0).reshape(E, BPC, H)
        hl_full[c * BPC : (c + 1) * BPC] = tl.transpose(1, 0, 2)
    return h_full, hl_full
